# revision 9
# baseline (speedup 1.0000x reference)
"""VQ codebook nearest-neighbor kernel for Trainium2 (8 NeuronCores, SPMD).

v9: fp16 matmul + global (chunk-256) in-PSUM payload; the code index is
recovered directly from the low bits of the per-point min value, so there is
no level-2 machinery at all.  The distance chain is split into TWO matmul
instructions accumulating on the same PSUM region (HW-verified: a matmul
instruction's internal sum is wide/tree-combined and rounds ONCE at the PSUM
write, so every term that must round at a controlled magnitude needs its own
instruction):
  1. A (126 rows) per tile: z-products + (e^2/2 + B) split; its write lands
     in [8192,16384) where fp32 ulp = 2^-10, quantizing w to that grid.
  2. -B alone (1 row, per supergroup): exact recenter, stays on the grid.
  3. pay alone (1 row, per supergroup): deposits (k-128)*2^-18 exactly in the
     sub-grid bits (ulp(|w|<64) = 2^-18).
Putting -B and pay in one instruction annihilates pay (their internal sum
rounds at |B|); same for a single 128-row instruction (z partials + pay
combine at small magnitude with arbitrary low bits).

Problem: z [16, 64, 128, 128] f32, emb [256, 64] f32 ->
         codes [16, 128, 128] int32 = argmin_k ||x_p - emb_k||_2
         (x = z rearranged 't a b c -> t (b c) a').

Math (per point p, code k), all scaled by 1/2 so |w| < 64:
  w_k = x.(-emb^T) + |e_k|^2/2   (argmin_k w = argmin_k dist)
Single 128-row fp16 matmul per 128-point tile, moving rows in order:
  A rows 0-63:    Eh = fp16(-emb^T)       x  zh = fp16(z)
  A rows 64-122:  Eh[0:59]                x  zm = fp16(z - zh) (dims 0-58)
  A rows 123-125: a1,a2,a3 = exact 3-term fp16 split of (|e|^2/2 + B) x ones
  BC row 0:       -B                      x  ones
  BC row 1:       pay_k = (k-128)*2^-18   x  ones
with B = 12288 (psum in [8192,16384), ulp 2^-10).  vmin = min_k(w_q + pay_k)
recovers
  code = (int32(vmin * 2^18) + 128) & 255
exactly, with ties resolving to the smallest k (payload is increasing in k),
matching jnp.argmin.

Engines: PE 256 matmuls (~27us), DVE 32 tensor_reduce of 2048 f32 from PSUM
(~70us, the bottleneck - DVE is the only engine that can do mins in this
toolchain), Act does the extraction copy, DMA ~25us in.  All overlap.
"""

import sys

for _p in ("/opt/trn_rl_repo", "/root/.axon_site/_ro/trn_rl_repo"):
    if _p not in sys.path:
        sys.path.insert(0, _p)

import numpy as np

import concourse.bass as bass
import concourse.bacc as bacc
import concourse.mybir as mybir
from concourse import tile
from concourse.bass_utils import run_bass_kernel_spmd

F32 = mybir.dt.float32
FP16 = mybir.dt.float16
I32 = mybir.dt.int32

N_CORES = 8
T_TOTAL = 16
N_SLICES = T_TOTAL // N_CORES          # t-slices per core (2)
POINTS = 128 * 128                     # points per t-slice
TILES = POINTS // 128                  # 128 tiles of 128 points
N_CHUNKS = 4                           # DMA chunks per slice
CHUNK_PTS = POINTS // N_CHUNKS         # 4096
TILES_PER_CHUNK = CHUNK_PTS // 128     # 32
SG_TILES = 8                           # tiles per PSUM supergroup
K = 256
D = 64
D_ZM = 59                              # zm dims kept (59 + 64 + 5 consts = 128)

B_BIG = 12288.0                        # w/2+B in [8192,16384) -> quantum 2^-10
PAYSCALE = 262144.0                    # 2^18

AluOp = mybir.AluOpType
Axis = mybir.AxisListType
ActFn = mybir.ActivationFunctionType


def _build_nc():
    nc = bacc.Bacc(None, target_bir_lowering=False, debug=False)

    # packed stationary planes: rows 0-63 zh, 64-122 zm[0:59], 123-125 ones
    zpa_d = nc.declare_dram_parameter(
        "zpa", [N_SLICES, 126, POINTS], FP16, isOutput=False
    )
    mov_d = nc.declare_dram_parameter("mov", [128, K], FP16, isOutput=False)
    movb_d = nc.declare_dram_parameter("movb", [1, K], FP16, isOutput=False)
    movp_d = nc.declare_dram_parameter("movp", [1, K], FP16, isOutput=False)
    onesst_d = nc.declare_dram_parameter("onesst", [1, 128], FP16, isOutput=False)
    codes_d = nc.declare_dram_parameter(
        "codes", [N_SLICES, 128, TILES], I32, isOutput=True
    )

    with tile.TileContext(nc) as tc:
        with (
            tc.tile_pool(name="cst", bufs=1) as cst_pool,
            tc.tile_pool(name="chunk", bufs=3) as chunk_pool,
            tc.tile_pool(name="psum", bufs=2, space="PSUM") as psum_pool,
            tc.tile_pool(name="vmin", bufs=2) as vmin_pool,
            tc.tile_pool(name="ex", bufs=2) as ex_pool,
        ):
            mov = cst_pool.tile([128, K], FP16, tag="mov")
            movb = cst_pool.tile([1, K], FP16, tag="movb")
            movp = cst_pool.tile([1, K], FP16, tag="movp")
            onesst = cst_pool.tile([1, 128], FP16, tag="onesst")
            nc.sync.dma_start(mov[:], mov_d[:])
            nc.sync.dma_start(movb[:], movb_d[:])
            nc.sync.dma_start(movp[:], movp_d[:])
            nc.sync.dma_start(onesst[:], onesst_d[:])

            for s in range(N_SLICES):
                vmin_sb = vmin_pool.tile([128, TILES], F32)
                for c in range(N_CHUNKS):
                    ch = chunk_pool.tile([126, CHUNK_PTS], FP16, tag="ch")
                    rng = slice(c * CHUNK_PTS, (c + 1) * CHUNK_PTS)
                    nc.sync.dma_start(ch[:], zpa_d[s, :, rng])
                    for g in range(TILES_PER_CHUNK // SG_TILES):
                        ps = psum_pool.tile([128, SG_TILES * K], F32)
                        for j in range(SG_TILES):
                            t_loc = g * SG_TILES + j      # tile within chunk
                            cols = slice(128 * t_loc, 128 * (t_loc + 1))
                            # start=True resets the WHOLE 2KB psum bank (2
                            # tiles), so only the first matmul per bank sets it
                            nc.tensor.matmul(
                                ps[:, K * j : K * (j + 1)],
                                ch[:, cols],
                                mov[0:126, :],
                                start=(j % 2 == 0),
                                stop=False,
                                skip_group_check=True,
                            )
                        # -B then pay per tile, each a single-row matmul
                        # (separate instructions: see header)
                        for j in range(SG_TILES):
                            reg = ps[:, K * j : K * (j + 1)]
                            nc.tensor.matmul(
                                reg,
                                onesst[:],
                                movb[:],
                                start=False,
                                stop=False,
                                skip_group_check=True,
                            )
                            nc.tensor.matmul(
                                reg,
                                onesst[:],
                                movp[:],
                                start=False,
                                stop=(j % 2 == 1),
                                skip_group_check=True,
                            )
                        t0 = c * TILES_PER_CHUNK + g * SG_TILES
                        nc.vector.tensor_reduce(
                            vmin_sb[:, t0 : t0 + SG_TILES],
                            ps[:].rearrange("p (j k) -> p j k", j=SG_TILES),
                            axis=Axis.X,
                            op=AluOp.min,
                        )
                # extraction: code = (int32(vmin*2^17) + 128) & 255
                tex = ex_pool.tile([128, TILES], I32, tag="tex")
                nc.scalar.activation(
                    tex[:], vmin_sb[:], ActFn.Copy, bias=128.0, scale=PAYSCALE
                )
                cod = ex_pool.tile([128, TILES], I32, tag="cod")
                nc.vector.tensor_scalar(
                    cod[:], tex[:], 255, None, AluOp.bitwise_and
                )
                nc.sync.dma_start(codes_d[s], cod[:])
    nc.compile()
    return nc


def _make_mov(emb: np.ndarray):
    e2 = (emb.astype(np.float64) ** 2).sum(axis=-1)          # [256]
    E = (-1.0 * emb.T).astype(np.float32)                    # [64, 256] = -emb^T
    Eh = E.astype(np.float16)

    # 3-term fp16 split of (e2/2 + B)
    v = e2 * 0.5 + B_BIG
    a1 = v.astype(np.float32).astype(np.float16)
    a2 = (v - a1.astype(np.float64)).astype(np.float32).astype(np.float16)
    a3 = (
        (v - a1.astype(np.float64) - a2.astype(np.float64))
        .astype(np.float32)
        .astype(np.float16)
    )
    pay = ((np.arange(K) - 128.0) * 2.0**-18).astype(np.float16)

    mov = np.zeros((128, K), dtype=np.float16)
    mov[0:D] = Eh
    mov[D : D + D_ZM] = Eh[0:D_ZM]
    mov[123] = a1
    mov[124] = a2
    mov[125] = a3
    movb = np.full((1, K), -B_BIG, dtype=np.float16)
    movp = pay[None, :]
    onesst = np.ones((1, 128), dtype=np.float16)
    return mov, movb, movp, onesst


def _pack_z(zr: np.ndarray):
    """zr [S, 64, POINTS] f32 -> zpa [S, 126, POINTS] fp16.

    rows 0-63 zh, 64-122 zm[0:59], 123-125 ones."""
    S, _, P = zr.shape
    zh = zr.astype(np.float16)
    zm = (zr - zh.astype(np.float32)).astype(np.float16)
    zpa = np.ones((S, 126, P), dtype=np.float16)
    zpa[:, 0:D] = zh
    zpa[:, D : D + D_ZM] = zm[:, 0:D_ZM]
    return zpa


def _run(z: np.ndarray, emb: np.ndarray, **spmd_kwargs):
    z = np.asarray(z, dtype=np.float32)
    emb = np.asarray(emb, dtype=np.float32)
    t, a, b, c = z.shape
    assert (t, a, b, c) == (16, 64, 128, 128) and emb.shape == (256, 64)

    zr = z.reshape(t, a, b * c)
    mov, movb, movp, onesst = _make_mov(emb)

    nc = _build_nc()
    in_maps = []
    for i in range(N_CORES):
        zpa = _pack_z(zr[i * N_SLICES : (i + 1) * N_SLICES])
        in_maps.append(
            {"zpa": zpa, "mov": mov, "movb": movb, "movp": movp, "onesst": onesst}
        )
    res = run_bass_kernel_spmd(
        nc, in_maps, core_ids=list(range(N_CORES)), **spmd_kwargs
    )

    out = np.empty((t, b * c), dtype=np.int32)
    for i in range(N_CORES):
        arr = np.asarray(res.results[i]["codes"])   # [N_SLICES, 128, TILES]
        # point = 128*tile + partition -> codes[s, p, j] -> out[point] = arr[s].T
        out[i * N_SLICES : (i + 1) * N_SLICES] = (
            arr.transpose(0, 2, 1).reshape(N_SLICES, b * c).astype(np.int32)
        )
    return out.reshape(t, b, c), res


def kernel(z: np.ndarray, emb: np.ndarray) -> np.ndarray:
    return _run(z, emb)[0]


# revision 11
# speedup vs baseline: 1.0540x; 1.0540x over previous
"""VQ codebook nearest-neighbor kernel for Trainium2 (8 NeuronCores, SPMD).

v9: fp16 matmul + global (chunk-256) in-PSUM payload; the code index is
recovered directly from the low bits of the per-point min value, so there is
no level-2 machinery at all.  The distance chain is split into TWO matmul
instructions accumulating on the same PSUM region (HW-verified: a matmul
instruction's internal sum is wide/tree-combined and rounds ONCE at the PSUM
write, so every term that must round at a controlled magnitude needs its own
instruction):
  1. A (126 rows) per tile: z-products + (e^2/2 + B) split; its PSUM write
     rounds at [8192,16384) magnitude where fp32 ulp = 2^-10, quantizing w
     to that grid.  (start=True resets the whole 2KB bank, so only the first
     A per bank sets it.)
  2. ScalarE recenters in place: activation Copy(psum*1 + (-B)) -> psum,
     exact (both operands on the 2^-10 grid) - zero PE columns spent.
  3. pay alone (1 row, 512-col per bank): deposits (k-128)*2^-18 exactly in
     the sub-grid bits (ulp(|w|<64) = 2^-18).
Putting -B and pay in one instruction annihilates pay (their internal sum
rounds at |B|); same for a single 128-row instruction (z partials + pay
combine at small magnitude with arbitrary low bits).

Problem: z [16, 64, 128, 128] f32, emb [256, 64] f32 ->
         codes [16, 128, 128] int32 = argmin_k ||x_p - emb_k||_2
         (x = z rearranged 't a b c -> t (b c) a').

Math (per point p, code k), all scaled by 1/2 so |w| < 64:
  w_k = x.(-emb^T) + |e_k|^2/2   (argmin_k w = argmin_k dist)
Single 128-row fp16 matmul per 128-point tile, moving rows in order:
  A rows 0-63:    Eh = fp16(-emb^T)       x  zh = fp16(z)
  A rows 64-122:  Eh[0:59]                x  zm = fp16(z - zh) (dims 0-58)
  A rows 123-125: a1,a2,a3 = exact 3-term fp16 split of (|e|^2/2 + B) x ones
  BC row 0:       -B                      x  ones
  BC row 1:       pay_k = (k-128)*2^-18   x  ones
with B = 12288 (psum in [8192,16384), ulp 2^-10).  vmin = min_k(w_q + pay_k)
recovers
  code = (int32(vmin * 2^18) + 128) & 255
exactly, with ties resolving to the smallest k (payload is increasing in k),
matching jnp.argmin.

Engines: PE 256 matmuls (~27us), DVE 32 tensor_reduce of 2048 f32 from PSUM
(~70us, the bottleneck - DVE is the only engine that can do mins in this
toolchain), Act does the extraction copy, DMA ~25us in.  All overlap.
"""

import sys

for _p in ("/opt/trn_rl_repo", "/root/.axon_site/_ro/trn_rl_repo"):
    if _p not in sys.path:
        sys.path.insert(0, _p)

import numpy as np

import concourse.bass as bass
import concourse.bacc as bacc
import concourse.mybir as mybir
from concourse import tile
from concourse.bass_utils import run_bass_kernel_spmd

F32 = mybir.dt.float32
FP16 = mybir.dt.float16
I32 = mybir.dt.int32

N_CORES = 8
T_TOTAL = 16
N_SLICES = T_TOTAL // N_CORES          # t-slices per core (2)
POINTS = 128 * 128                     # points per t-slice
TILES = POINTS // 128                  # 128 tiles of 128 points
N_CHUNKS = 4                           # DMA chunks per slice
CHUNK_PTS = POINTS // N_CHUNKS         # 4096
TILES_PER_CHUNK = CHUNK_PTS // 128     # 32
SG_TILES = 8                           # tiles per PSUM supergroup
K = 256
D = 64
D_ZM = 59                              # zm dims kept (59 + 64 + 5 consts = 128)

B_BIG = 12288.0                        # w/2+B in [8192,16384) -> quantum 2^-10
PAYSCALE = 262144.0                    # 2^18

AluOp = mybir.AluOpType
Axis = mybir.AxisListType
ActFn = mybir.ActivationFunctionType


def _build_nc():
    nc = bacc.Bacc(None, target_bir_lowering=False, debug=False)

    # packed stationary planes: rows 0-63 zh, 64-122 zm[0:59], 123-125 ones
    zpa_d = nc.declare_dram_parameter(
        "zpa", [N_SLICES, 126, POINTS], FP16, isOutput=False
    )
    mov_d = nc.declare_dram_parameter("mov", [128, K], FP16, isOutput=False)
    movp_d = nc.declare_dram_parameter("movp", [1, 2 * K], FP16, isOutput=False)
    onesst_d = nc.declare_dram_parameter("onesst", [1, 128], FP16, isOutput=False)
    codes_d = nc.declare_dram_parameter(
        "codes", [N_SLICES, 128, TILES], I32, isOutput=True
    )

    with tile.TileContext(nc) as tc:
        with (
            tc.tile_pool(name="cst", bufs=1) as cst_pool,
            tc.tile_pool(name="chunk", bufs=3) as chunk_pool,
            tc.tile_pool(name="psum", bufs=2, space="PSUM") as psum_pool,
            tc.tile_pool(name="vmin", bufs=2) as vmin_pool,
            tc.tile_pool(name="ex", bufs=2) as ex_pool,
        ):
            mov = cst_pool.tile([128, K], FP16, tag="mov")
            movp = cst_pool.tile([1, 2 * K], FP16, tag="movp")
            onesst = cst_pool.tile([1, 128], FP16, tag="onesst")
            nc.sync.dma_start(mov[:], mov_d[:])
            nc.sync.dma_start(movp[:], movp_d[:])
            nc.sync.dma_start(onesst[:], onesst_d[:])

            for s in range(N_SLICES):
                vmin_sb = vmin_pool.tile([128, TILES], F32)
                for c in range(N_CHUNKS):
                    ch = chunk_pool.tile([126, CHUNK_PTS], FP16, tag="ch")
                    rng = slice(c * CHUNK_PTS, (c + 1) * CHUNK_PTS)
                    nc.sync.dma_start(ch[:], zpa_d[s, :, rng])
                    for g in range(TILES_PER_CHUNK // SG_TILES):
                        ps = psum_pool.tile([128, SG_TILES * K], F32)
                        for j in range(SG_TILES):
                            t_loc = g * SG_TILES + j      # tile within chunk
                            cols = slice(128 * t_loc, 128 * (t_loc + 1))
                            # start=True resets the WHOLE 2KB psum bank
                            nc.tensor.matmul(
                                ps[:, K * j : K * (j + 1)],
                                ch[:, cols],
                                mov[0:126, :],
                                start=(j % 2 == 0),
                                stop=False,
                                skip_group_check=True,
                            )
                        # ScalarE in-place recenter: psum += -B (exact)
                        nc.scalar.activation(
                            ps[:], ps[:], ActFn.Copy, bias=-B_BIG, scale=1.0
                        )
                        # pay finisher: one single-row 512-col matmul per bank
                        for q in range(SG_TILES // 2):
                            nc.tensor.matmul(
                                ps[:, 2 * K * q : 2 * K * (q + 1)],
                                onesst[:],
                                movp[:],
                                start=False,
                                stop=True,
                                skip_group_check=True,
                            )
                        t0 = c * TILES_PER_CHUNK + g * SG_TILES
                        nc.vector.tensor_reduce(
                            vmin_sb[:, t0 : t0 + SG_TILES],
                            ps[:].rearrange("p (j k) -> p j k", j=SG_TILES),
                            axis=Axis.X,
                            op=AluOp.min,
                        )
                # extraction: code = (int32(vmin*2^17) + 128) & 255
                tex = ex_pool.tile([128, TILES], I32, tag="tex")
                nc.scalar.activation(
                    tex[:], vmin_sb[:], ActFn.Copy, bias=128.0, scale=PAYSCALE
                )
                cod = ex_pool.tile([128, TILES], I32, tag="cod")
                nc.vector.tensor_scalar(
                    cod[:], tex[:], 255, None, AluOp.bitwise_and
                )
                nc.sync.dma_start(codes_d[s], cod[:])
    nc.compile()
    return nc


def _make_mov(emb: np.ndarray):
    e2 = (emb.astype(np.float64) ** 2).sum(axis=-1)          # [256]
    E = (-1.0 * emb.T).astype(np.float32)                    # [64, 256] = -emb^T
    Eh = E.astype(np.float16)

    # 3-term fp16 split of (e2/2 + B)
    v = e2 * 0.5 + B_BIG
    a1 = v.astype(np.float32).astype(np.float16)
    a2 = (v - a1.astype(np.float64)).astype(np.float32).astype(np.float16)
    a3 = (
        (v - a1.astype(np.float64) - a2.astype(np.float64))
        .astype(np.float32)
        .astype(np.float16)
    )
    pay = ((np.arange(K) - 128.0) * 2.0**-18).astype(np.float16)

    mov = np.zeros((128, K), dtype=np.float16)
    mov[0:D] = Eh
    mov[D : D + D_ZM] = Eh[0:D_ZM]
    mov[123] = a1
    mov[124] = a2
    mov[125] = a3
    movp = np.tile(pay, 2)[None, :]
    onesst = np.ones((1, 128), dtype=np.float16)
    return mov, movp, onesst


def _pack_z(zr: np.ndarray):
    """zr [S, 64, POINTS] f32 -> zpa [S, 126, POINTS] fp16.

    rows 0-63 zh, 64-122 zm[0:59], 123-125 ones."""
    S, _, P = zr.shape
    zh = zr.astype(np.float16)
    zm = (zr - zh.astype(np.float32)).astype(np.float16)
    zpa = np.ones((S, 126, P), dtype=np.float16)
    zpa[:, 0:D] = zh
    zpa[:, D : D + D_ZM] = zm[:, 0:D_ZM]
    return zpa


def _run(z: np.ndarray, emb: np.ndarray, **spmd_kwargs):
    z = np.asarray(z, dtype=np.float32)
    emb = np.asarray(emb, dtype=np.float32)
    t, a, b, c = z.shape
    assert (t, a, b, c) == (16, 64, 128, 128) and emb.shape == (256, 64)

    zr = z.reshape(t, a, b * c)
    mov, movp, onesst = _make_mov(emb)

    nc = _build_nc()
    in_maps = []
    for i in range(N_CORES):
        zpa = _pack_z(zr[i * N_SLICES : (i + 1) * N_SLICES])
        in_maps.append(
            {"zpa": zpa, "mov": mov, "movp": movp, "onesst": onesst}
        )
    res = run_bass_kernel_spmd(
        nc, in_maps, core_ids=list(range(N_CORES)), **spmd_kwargs
    )

    out = np.empty((t, b * c), dtype=np.int32)
    for i in range(N_CORES):
        arr = np.asarray(res.results[i]["codes"])   # [N_SLICES, 128, TILES]
        # point = 128*tile + partition -> codes[s, p, j] -> out[point] = arr[s].T
        out[i * N_SLICES : (i + 1) * N_SLICES] = (
            arr.transpose(0, 2, 1).reshape(N_SLICES, b * c).astype(np.int32)
        )
    return out.reshape(t, b, c), res


def kernel(z: np.ndarray, emb: np.ndarray) -> np.ndarray:
    return _run(z, emb)[0]


# revision 12
# speedup vs baseline: 1.4174x; 1.3449x over previous
"""VQ codebook nearest-neighbor kernel for Trainium2 (8 NeuronCores, SPMD).

v5: 2-pass fp16 matmuls + in-PSUM payload; no elementwise payload plumbing.
Super-groups of 4 tiles share a 2-bank PSUM tile; the payload pass is a
shape-uniform [128,128]x[128,1024] matmul (row 127 = ones x pay row, other
moving rows zero) so the PE never reconfigures tile size; stage-1 reduce is
batched over 4 tiles; eq/bs of level-2 run on GpSimd.

Problem: z [16, 64, 128, 128] f32, emb [256, 64] f32 ->
         codes [16, 128, 128] int32 = argmin_k ||x_p - emb_k||_2
         (x = z rearranged 't a b c -> t (b c) a').

Per 128-point tile (fp16 splits: z = zh + zm, E = -2 emb^T = Eh + Em):
  P1 [68 rows]:  zh.Eh + (|e|^2 + B_BIG)   (consts as exact 3-term fp16
                 split; PSUM stores ~3072+s, fp32 ulp 2^-12 = quantizer)
  P2 [128 rows]: zm[0:63].Eh + zh.Em - B_BIG
                 (-B is the LAST moving row: the chain rounds once at
                 2^-12 and PSUM recenters to w_q exactly; 1 zm dim dropped
                 to fit 128 rows, error ~1 quantum)
  P3 [rank-1, 512 cols/group]: + (k%16)*2^-16  (fp16 subnormals are exact;
                 chain stays small so the payload survives; PSUM now holds
                 w_q + payload for both tiles)
Stage 1: one DVE reduce_min over 16-wide k-chunks straight from PSUM.
Level 2 as v1: global min, equality bitmask dotted with 2^j,
lowest-set-bit -> chunk j*; payload of the min -> i*; code = 16*j* + i*.
Ties resolve to the smallest k, matching jnp.argmin.
"""

import sys

for _p in ("/opt/trn_rl_repo", "/root/.axon_site/_ro/trn_rl_repo"):
    if _p not in sys.path:
        sys.path.insert(0, _p)

import numpy as np

import concourse.bass as bass
import concourse.bacc as bacc
import concourse.mybir as mybir
from concourse import tile
from concourse.bass_utils import run_bass_kernel_spmd

F32 = mybir.dt.float32
FP16 = mybir.dt.float16
I32 = mybir.dt.int32

N_CORES = 8
T_TOTAL = 16
N_SLICES = T_TOTAL // N_CORES          # t-slices per core
POINTS = 128 * 128                     # points per t-slice
N_CHUNKS = 4                           # point-chunks per slice
CHUNK_PTS = POINTS // N_CHUNKS         # 4096
TILES_PER_CHUNK = CHUNK_PTS // 128     # 32
K = 256
D = 64

B_BIG = 3072.0                         # w+B in [2048,4096) -> quantum 2^-12
DELTA = 2.0 ** -16                     # payload step for i = k % 16
PAYSCALE = 2.0 ** 16

AluOp = mybir.AluOpType
Axis = mybir.AxisListType


def _build_nc():
    nc = bacc.Bacc(None, target_bir_lowering=False, debug=False)

    # A-plane: rows 0-62 zm (dim 63 dropped), 63-126 zh, 127 ones
    zpa_d = nc.declare_dram_parameter(
        "zpa", [N_SLICES, 128, POINTS], FP16, isOutput=False
    )
    # B-plane: rows 0-63 zh, 64-67 ones
    zpb_d = nc.declare_dram_parameter(
        "zpb", [N_SLICES, D + 4, POINTS], FP16, isOutput=False
    )
    mov1_d = nc.declare_dram_parameter("mov1", [D + 4, K], FP16, isOutput=False)
    mov2_d = nc.declare_dram_parameter("mov2", [128, K], FP16, isOutput=False)
    pay_d = nc.declare_dram_parameter("payrow", [1, 4 * K], FP16, isOutput=False)
    pow2_d = nc.declare_dram_parameter("pow2", [128, 16], F32, isOutput=False)
    codes_d = nc.declare_dram_parameter(
        "codes", [N_SLICES, 128, N_CHUNKS, TILES_PER_CHUNK], I32, isOutput=True
    )

    with tile.TileContext(nc) as tc:
        with (
            tc.tile_pool(name="cst", bufs=1) as cst_pool,
            tc.tile_pool(name="chunk", bufs=3) as chunk_pool,
            tc.tile_pool(name="psum", bufs=8, space="PSUM") as psum_pool,
            tc.tile_pool(name="m16", bufs=3) as m16_pool,
            tc.tile_pool(name="l2", bufs=2) as l2_pool,
            tc.tile_pool(name="codes", bufs=2) as codes_pool,
        ):
            mov1 = cst_pool.tile([D + 4, K], FP16, tag="mov1")
            mov2 = cst_pool.tile([128, K], FP16, tag="mov2")
            # payload moving tile: rows 0-126 zero, row 127 = pay pattern;
            # pairs with chA whose row 127 is ones -> shape-uniform matmul.
            paymt = cst_pool.tile([128, 4 * K], FP16, tag="paymt")
            nc.vector.memset(paymt[0:127, :], 0.0)
            payrow = paymt[127:128, 0 : 4 * K]
            pow2 = cst_pool.tile([128, 16], F32, tag="pow2")
            nc.sync.dma_start(mov1[:], mov1_d[:])
            nc.sync.dma_start(mov2[:], mov2_d[:])
            nc.sync.dma_start(payrow, pay_d[:])
            nc.sync.dma_start(pow2[:], pow2_d[:])

            for s in range(N_SLICES):
                codes_sb = codes_pool.tile([128, N_CHUNKS * TILES_PER_CHUNK], I32)
                for c in range(N_CHUNKS):
                    chA = chunk_pool.tile([128, CHUNK_PTS], FP16, tag="chA")
                    chB = chunk_pool.tile([D + 4, CHUNK_PTS], FP16, tag="chB")
                    rng = slice(c * CHUNK_PTS, (c + 1) * CHUNK_PTS)
                    nc.sync.dma_start(chA[:], zpa_d[s, :, rng])
                    nc.sync.dma_start(chB[:], zpb_d[s, :, rng])
                    # [*, 32, 128]: [:, j, m] = point 32*m + j
                    av = chA[:].rearrange("p (n j) -> p j n", j=TILES_PER_CHUNK)
                    bv = chB[:].rearrange("p (n j) -> p j n", j=TILES_PER_CHUNK)
                    m16 = m16_pool.tile([128, TILES_PER_CHUNK, 16], F32)
                    for p in range(TILES_PER_CHUNK // 2):
                        ps = psum_pool.tile([128, 512], F32)
                        for h in range(2):
                            j = 2 * p + h
                            reg = ps[:, 256 * h : 256 * (h + 1)]
                            nc.tensor.matmul(
                                reg,
                                bv[:, j, :],
                                mov1[:],
                                start=(h == 0),
                                stop=False,
                                skip_group_check=True,
                            )
                            nc.tensor.matmul(
                                reg,
                                av[:, j, :],
                                mov2[:],
                                start=False,
                                stop=False,
                                skip_group_check=True,
                            )
                        # payload pass over both tiles; shape-uniform matmul
                        # (stationary = chA slice already in SBUF, row 127 = ones;
                        # moving rows 0-126 are zero, row 127 = pay pattern)
                        nc.tensor.matmul(
                            ps[:, 0:512],
                            av[:, 2 * p, :],
                            paymt[:, 0:512],
                            start=False,
                            stop=True,
                            skip_group_check=True,
                        )
                        nc.vector.tensor_reduce(
                            m16[:, 2 * p : 2 * p + 2, :],
                            ps[:].rearrange("m (t c i) -> m t c i", t=2, c=16, i=16),
                            axis=Axis.X,
                            op=AluOp.min,
                        )

                    # ---- level 2: 32 tiles at once ----
                    vmin = l2_pool.tile([128, TILES_PER_CHUNK], F32)
                    nc.vector.tensor_reduce(vmin[:], m16[:], axis=Axis.X, op=AluOp.min)
                    eq = l2_pool.tile([128, TILES_PER_CHUNK, 16], F32)
                    nc.vector.tensor_tensor(
                        eq[:],
                        m16[:],
                        vmin[:].unsqueeze(2).broadcast_to([128, TILES_PER_CHUNK, 16]),
                        op=AluOp.is_equal,
                    )
                    bs = l2_pool.tile([128, TILES_PER_CHUNK, 16], F32)
                    nc.vector.tensor_tensor(
                        bs[:],
                        eq[:],
                        pow2[:].unsqueeze(1).broadcast_to([128, TILES_PER_CHUNK, 16]),
                        op=AluOp.mult,
                    )
                    b = l2_pool.tile([128, TILES_PER_CHUNK], F32)
                    nc.vector.tensor_reduce(b[:], bs[:], axis=Axis.X, op=AluOp.add)
                    bi32 = l2_pool.tile([128, TILES_PER_CHUNK], I32)
                    nc.vector.tensor_copy(bi32[:], b[:])
                    nbi = l2_pool.tile([128, TILES_PER_CHUNK], I32)
                    nc.vector.tensor_scalar(nbi[:], b[:], -1.0, None, AluOp.mult)
                    low = l2_pool.tile([128, TILES_PER_CHUNK], I32)
                    nc.vector.tensor_tensor(low[:], bi32[:], nbi[:], op=AluOp.bitwise_and)
                    lowf = l2_pool.tile([128, TILES_PER_CHUNK], F32)
                    nc.vector.tensor_copy(lowf[:], low[:])
                    # j* = (float_bits(2^j) >> 23) - 127 ; jv = 16*j*
                    jt = l2_pool.tile([128, TILES_PER_CHUNK], I32)
                    nc.vector.tensor_scalar(
                        jt[:], lowf[:].bitcast(I32), 23, None, AluOp.arith_shift_right
                    )
                    jv = l2_pool.tile([128, TILES_PER_CHUNK], I32)
                    nc.vector.tensor_scalar(
                        jv[:], jt[:], 127, 16, AluOp.subtract, AluOp.mult
                    )
                    # i* = (int(vmin * 2^16)) & 15
                    t1 = l2_pool.tile([128, TILES_PER_CHUNK], I32)
                    nc.vector.tensor_scalar(t1[:], vmin[:], PAYSCALE, None, AluOp.mult)
                    t2 = l2_pool.tile([128, TILES_PER_CHUNK], I32)
                    nc.vector.tensor_scalar(t2[:], t1[:], 15, None, AluOp.bitwise_and)
                    nc.vector.tensor_tensor(
                        codes_sb[:, c * TILES_PER_CHUNK : (c + 1) * TILES_PER_CHUNK],
                        jv[:],
                        t2[:],
                        op=AluOp.add,
                    )
                nc.sync.dma_start(
                    codes_d[s],
                    codes_sb[:].rearrange("m (c j) -> m c j", c=N_CHUNKS),
                )
    nc.compile()
    return nc


def _make_consts(emb: np.ndarray):
    e2 = (emb.astype(np.float64) ** 2).sum(axis=-1)
    E = (-2.0 * emb.T).astype(np.float32)          # [64, 256]
    Eh = E.astype(np.float16)
    Em = (E - Eh.astype(np.float32)).astype(np.float16)

    v = e2 + B_BIG
    a1 = v.astype(np.float32).astype(np.float16)
    a2 = (v - a1.astype(np.float64)).astype(np.float32).astype(np.float16)
    a3 = (v - a1.astype(np.float64) - a2.astype(np.float64)).astype(
        np.float32
    ).astype(np.float16)

    mov1 = np.zeros((D + 4, K), dtype=np.float16)
    mov1[0:D] = Eh
    mov1[D] = a1
    mov1[D + 1] = a2
    mov1[D + 2] = a3

    mov2 = np.empty((128, K), dtype=np.float16)
    mov2[0:63] = Eh[0:63]
    mov2[63:127] = Em
    mov2[127] = np.float16(-B_BIG)

    payrow = ((np.arange(4 * K) % 16).astype(np.float32) * DELTA).astype(
        np.float16
    )[None, :]
    pow2 = np.broadcast_to(
        (2.0 ** np.arange(16)).astype(np.float32), (128, 16)
    ).copy()
    return mov1, mov2, payrow, pow2


def _pack_z(zr: np.ndarray):
    """zr [S, 64, POINTS] f32 -> (zpa [S,128,POINTS], zpb [S,68,POINTS]) fp16."""
    S, _, P = zr.shape
    zh = zr.astype(np.float16)
    zm = (zr - zh.astype(np.float32)).astype(np.float16)
    zpa = np.empty((S, 128, P), dtype=np.float16)
    zpa[:, 0:63] = zm[:, 0:63]
    zpa[:, 63:127] = zh
    zpa[:, 127] = np.float16(1.0)
    zpb = np.ones((S, D + 4, P), dtype=np.float16)
    zpb[:, 0:D] = zh
    return zpa, zpb


def _run(z: np.ndarray, emb: np.ndarray, **spmd_kwargs):
    z = np.asarray(z, dtype=np.float32)
    emb = np.asarray(emb, dtype=np.float32)
    t, a, b, c = z.shape
    assert (t, a, b, c) == (16, 64, 128, 128) and emb.shape == (256, 64)

    zr = z.reshape(t, a, b * c)
    mov1, mov2, payrow, pow2 = _make_consts(emb)

    nc = _build_nc()
    in_maps = []
    for i in range(N_CORES):
        zpa, zpb = _pack_z(zr[i * N_SLICES : (i + 1) * N_SLICES])
        in_maps.append(
            {
                "zpa": zpa,
                "zpb": zpb,
                "mov1": mov1,
                "mov2": mov2,
                "payrow": payrow,
                "pow2": pow2,
            }
        )
    res = run_bass_kernel_spmd(nc, in_maps, core_ids=list(range(N_CORES)), **spmd_kwargs)

    out = np.empty((t, b * c), dtype=np.int32)
    for i in range(N_CORES):
        arr = np.asarray(res.results[i]["codes"])  # [N_SLICES, 128, N_CHUNKS, 32]
        # point p = 4096*c + 32*m + j  ->  [s, c, m, j] order is p-major
        out[i * N_SLICES : (i + 1) * N_SLICES] = (
            arr.transpose(0, 2, 1, 3).reshape(N_SLICES, b * c).astype(np.int32)
        )
    return out.reshape(t, b, c), res


def kernel(z: np.ndarray, emb: np.ndarray) -> np.ndarray:
    return _run(z, emb)[0]

